# revision 4
# baseline (speedup 1.0000x reference)
"""nn_CDIM cross-modality fusion forward pass on Trainium2 (axon-tunneled).

Strategy: the axon tunnel moves ~40-70 MB/s, so wall time is dominated by
host<->device transfer, not device compute. We therefore:
  - cast x,y and all weights to bf16 on host (halves upload bytes),
  - shard the batch (B=4) across 4 NeuronCores via shard_map (no collectives;
    the model has no cross-sample interaction),
  - compute everything on device in bf16 with f32 accumulation,
  - return a bf16 result and cast to f32 on host (halves download bytes).

Shapes are hardcoded for B=4, C=64, H=W=256 (see spec)."""

import os

os.environ["JAX_PLATFORMS"] = "axon,cpu"
os.environ.setdefault("NEURON_RT_VISIBLE_CORES", "")
os.environ.pop("NEURON_RT_VISIBLE_CORES", None)

import numpy as np

import jax
import jax.numpy as jnp
from jax import lax
from jax.sharding import Mesh, PartitionSpec as P, NamedSharding
from jax.experimental.shard_map import shard_map
from functools import partial

SIZE = 32
B, C, H, W = 4, 64, 256, 256
NDEV = 4


def _cubic_kernel(x):
    x = np.abs(x)
    out = ((1.5 * x - 2.5) * x) * x + 1.0
    out = np.where(x >= 1.0, ((-0.5 * x + 2.5) * x - 4.0) * x + 2.0, out)
    return np.where(x >= 2.0, 0.0, out)


def _resize_mat(in_size, out_size):
    inv_scale = in_size / out_size
    sample_f = (np.arange(out_size, dtype=np.float64) + 0.5) * inv_scale - 0.5
    x = sample_f[None, :] - np.arange(in_size, dtype=np.float64)[:, None]
    weights = _cubic_kernel(x)
    total = weights.sum(axis=0, keepdims=True)
    weights = np.where(
        np.abs(total) > 1000.0 * np.finfo(np.float32).eps,
        weights / np.where(total != 0, total, 1),
        0.0,
    )
    weights = np.where(
        (sample_f[None, :] >= -0.5) & (sample_f[None, :] <= in_size - 0.5),
        weights,
        0.0,
    )
    return weights.astype(np.float32)


_M_DOWN = _resize_mat(H, SIZE)  # [256, 32]
_M_UP = _resize_mat(SIZE, H)  # [32, 256]

_BF = jnp.bfloat16


def _resize(x, M):
    # x: [B, C, h, w] -> contract h then w with M (same matrix both dims).
    Mb = M.astype(_BF)
    t = jnp.einsum("bchw,hi->bciw", x, Mb, preferred_element_type=jnp.float32)
    t = jnp.einsum("bciw,wj->bcij", t.astype(_BF), Mb,
                   preferred_element_type=jnp.float32)
    return t.astype(_BF)


def _conv3x3(x, w, b=None):
    out = lax.conv_general_dilated(
        x, w, (1, 1), "SAME",
        dimension_numbers=("NCHW", "OIHW", "NCHW"),
        preferred_element_type=jnp.float32,
    )
    if b is not None:
        out = out + b[None, :, None, None].astype(jnp.float32)
    return out


def _bconv(x, w, b):
    return jax.nn.relu(_conv3x3(x, w, b)).astype(_BF)


def _spatial_attention(x, w):
    avg = jnp.mean(x.astype(jnp.float32), axis=1, keepdims=True)
    mx = jnp.max(x, axis=1, keepdims=True).astype(jnp.float32)
    # bf16-accumulated conv: the f32-accumulate conv fused with sigmoid trips
    # a TongaISel 'Unexpected cast!' assert in neuronx-cc. 18-term bf16
    # accumulation is well within tolerance here.
    a = lax.conv_general_dilated(
        jnp.concatenate([avg, mx], axis=1).astype(_BF), w, (1, 1), "SAME",
        dimension_numbers=("NCHW", "OIHW", "NCHW"),
    )
    return (jax.nn.sigmoid(a) * x + x).astype(_BF)


def _attention(Q, K, V, original, gamma, md_up):
    # Q, K, V: [b, C, S]
    E = jnp.einsum("bcs,bct->bst", K, Q, preferred_element_type=jnp.float32)
    mask = jax.nn.softmax(E, axis=-1).astype(_BF)
    refine = jnp.einsum("bcs,bts->bct", V, mask,
                        preferred_element_type=jnp.float32)
    refine = (gamma.astype(jnp.float32) * refine).astype(_BF)
    refine = refine.reshape(-1, C, SIZE, SIZE)
    up = jnp.einsum("bcij,ih->bchj", refine, md_up,
                    preferred_element_type=jnp.float32)
    up = jnp.einsum("bchj,jw->bchw", up.astype(_BF), md_up,
                    preferred_element_type=jnp.float32)
    return up.astype(_BF) + original


def _forward(x, y, wq1, bq1, wk1, bk1, wv1, bv1, wq2, bq2, wk2, bk2, wv2, bv2,
             w_reduce, b_reduce, w_sec, b_sec, w_sa_rgb, w_sa_inf,
             g1, g2, g3, g4):
    S = SIZE * SIZE
    md_up = jnp.asarray(_M_UP).astype(_BF)  # [32, 256]: in (i) -> out (h)
    x_re = _resize(x, jnp.asarray(_M_DOWN))
    y_re = _resize(y, jnp.asarray(_M_DOWN))
    n = x.shape[0]

    def qkv(inp, wq, bq, wk, bk, wv, bv):
        Q = _bconv(inp, wq, bq).reshape(n, C, S)
        K = _bconv(inp, wk, bk).reshape(n, C, S)
        V = _bconv(inp, wv, bv).reshape(n, C, S)
        return Q, K, V

    RQ, RK, RV = qkv(x_re, wq1, bq1, wk1, bk1, wv1, bv1)
    IQ, IK, IV = qkv(y_re, wq2, bq2, wk2, bk2, wv2, bv2)
    DV = (RV.astype(jnp.float32) + IV.astype(jnp.float32)).astype(_BF)

    r1 = _attention(RQ, RK, DV, x, g1, md_up)
    r2 = _attention(IQ, IK, DV, y, g2, md_up)
    r3 = _attention(RQ, IK, RV, y, g3, md_up)
    r4 = _attention(IQ, RK, IV, x, g4, md_up)

    glob = _bconv(jnp.concatenate([r1, r2, r3, r4], axis=1), w_reduce, b_reduce)
    sa_rgb = _spatial_attention(x, w_sa_rgb)
    sa_inf = _spatial_attention(y, w_sa_inf)
    out = _bconv(jnp.concatenate([glob, sa_inf, sa_rgb], axis=1), w_sec, b_sec)
    return out


_compiled = None


def _get_compiled():
    global _compiled
    if _compiled is not None:
        return _compiled
    devs = jax.devices()[:NDEV]
    mesh = Mesh(np.array(devs), ("b",))
    xspec = P("b")
    wspec = P()
    in_specs = (xspec, xspec) + (wspec,) * 22
    fn = jax.jit(
        shard_map(_forward, mesh=mesh, in_specs=in_specs, out_specs=xspec,
                  check_rep=False)
    )
    _compiled = (fn, mesh)
    return _compiled


def kernel(**inputs) -> np.ndarray:
    import ml_dtypes

    fn, mesh = _get_compiled()
    bf = ml_dtypes.bfloat16

    def cast(v):
        return np.asarray(v).astype(bf)

    a = {k: cast(v) for k, v in inputs.items()}
    args = (
        a["x"], a["y"],
        a["w_rgb_q"], a["b_rgb_q"], a["w_rgb_k"], a["b_rgb_k"],
        a["w_rgb_v"], a["b_rgb_v"],
        a["w_inf_q"], a["b_inf_q"], a["w_inf_k"], a["b_inf_k"],
        a["w_inf_v"], a["b_inf_v"],
        a["w_reduce"], a["b_reduce"], a["w_sec"], a["b_sec"],
        a["w_sa_rgb"], a["w_sa_inf"],
        a["gamma1"], a["gamma2"], a["gamma3"], a["gamma4"],
    )
    out = fn(*args)
    return np.asarray(out).astype(np.float32)


# revision 5
# speedup vs baseline: 1.1020x; 1.1020x over previous
"""nn_CDIM cross-modality fusion forward pass on Trainium2 (axon-tunneled).

Strategy: the axon tunnel moves ~40-70 MB/s, so wall time is dominated by
host<->device transfer, not device compute. We therefore:
  - cast x,y and all weights to bf16 on host (halves upload bytes),
  - shard the batch (B=4) across 4 NeuronCores via shard_map (no collectives;
    the model has no cross-sample interaction),
  - compute everything on device in bf16 with f32 accumulation,
  - return a bf16 result and cast to f32 on host (halves download bytes).

Shapes are hardcoded for B=4, C=64, H=W=256 (see spec)."""

import os

os.environ["JAX_PLATFORMS"] = "axon,cpu"
os.environ.setdefault("NEURON_RT_VISIBLE_CORES", "")
os.environ.pop("NEURON_RT_VISIBLE_CORES", None)

import numpy as np

import jax

jax.config.update("jax_compilation_cache_dir", "/root/.jax_cache")
jax.config.update("jax_persistent_cache_min_entry_size_bytes", -1)
jax.config.update("jax_persistent_cache_min_compile_time_secs", 0.0)

import jax.numpy as jnp
from jax import lax
from jax.sharding import Mesh, PartitionSpec as P, NamedSharding
from jax.experimental.shard_map import shard_map
from functools import partial

SIZE = 32
B, C, H, W = 4, 64, 256, 256
NDEV = 4


def _cubic_kernel(x):
    x = np.abs(x)
    out = ((1.5 * x - 2.5) * x) * x + 1.0
    out = np.where(x >= 1.0, ((-0.5 * x + 2.5) * x - 4.0) * x + 2.0, out)
    return np.where(x >= 2.0, 0.0, out)


def _resize_mat(in_size, out_size):
    inv_scale = in_size / out_size
    sample_f = (np.arange(out_size, dtype=np.float64) + 0.5) * inv_scale - 0.5
    x = sample_f[None, :] - np.arange(in_size, dtype=np.float64)[:, None]
    weights = _cubic_kernel(x)
    total = weights.sum(axis=0, keepdims=True)
    weights = np.where(
        np.abs(total) > 1000.0 * np.finfo(np.float32).eps,
        weights / np.where(total != 0, total, 1),
        0.0,
    )
    weights = np.where(
        (sample_f[None, :] >= -0.5) & (sample_f[None, :] <= in_size - 0.5),
        weights,
        0.0,
    )
    return weights.astype(np.float32)


_M_DOWN = _resize_mat(H, SIZE)  # [256, 32]
_M_UP = _resize_mat(SIZE, H)  # [32, 256]

_BF = jnp.bfloat16


def _resize(x, M):
    # x: [B, C, h, w] -> contract h then w with M (same matrix both dims).
    Mb = M.astype(_BF)
    t = jnp.einsum("bchw,hi->bciw", x, Mb, preferred_element_type=jnp.float32)
    t = jnp.einsum("bciw,wj->bcij", t.astype(_BF), Mb,
                   preferred_element_type=jnp.float32)
    return t.astype(_BF)


def _conv3x3(x, w, b=None):
    out = lax.conv_general_dilated(
        x, w, (1, 1), "SAME",
        dimension_numbers=("NCHW", "OIHW", "NCHW"),
        preferred_element_type=jnp.float32,
    )
    if b is not None:
        out = out + b[None, :, None, None].astype(jnp.float32)
    return out


def _bconv(x, w, b):
    return jax.nn.relu(_conv3x3(x, w, b)).astype(_BF)


def _spatial_attention(x, w):
    avg = jnp.mean(x.astype(jnp.float32), axis=1, keepdims=True)
    mx = jnp.max(x, axis=1, keepdims=True).astype(jnp.float32)
    # bf16-accumulated conv: the f32-accumulate conv fused with sigmoid trips
    # a TongaISel 'Unexpected cast!' assert in neuronx-cc. 18-term bf16
    # accumulation is well within tolerance here.
    a = lax.conv_general_dilated(
        jnp.concatenate([avg, mx], axis=1).astype(_BF), w, (1, 1), "SAME",
        dimension_numbers=("NCHW", "OIHW", "NCHW"),
    )
    return (jax.nn.sigmoid(a) * x + x).astype(_BF)


def _attention(Q, K, V, original, gamma, md_up):
    # Q, K, V: [b, C, S]
    E = jnp.einsum("bcs,bct->bst", K, Q, preferred_element_type=jnp.float32)
    mask = jax.nn.softmax(E, axis=-1).astype(_BF)
    refine = jnp.einsum("bcs,bts->bct", V, mask,
                        preferred_element_type=jnp.float32)
    refine = (gamma.astype(jnp.float32) * refine).astype(_BF)
    refine = refine.reshape(-1, C, SIZE, SIZE)
    up = jnp.einsum("bcij,ih->bchj", refine, md_up,
                    preferred_element_type=jnp.float32)
    up = jnp.einsum("bchj,jw->bchw", up.astype(_BF), md_up,
                    preferred_element_type=jnp.float32)
    return up.astype(_BF) + original


def _forward(x, y, wq1, bq1, wk1, bk1, wv1, bv1, wq2, bq2, wk2, bk2, wv2, bv2,
             w_reduce, b_reduce, w_sec, b_sec, w_sa_rgb, w_sa_inf,
             g1, g2, g3, g4):
    S = SIZE * SIZE
    md_up = jnp.asarray(_M_UP).astype(_BF)  # [32, 256]: in (i) -> out (h)
    x_re = _resize(x, jnp.asarray(_M_DOWN))
    y_re = _resize(y, jnp.asarray(_M_DOWN))
    n = x.shape[0]

    def qkv(inp, wq, bq, wk, bk, wv, bv):
        Q = _bconv(inp, wq, bq).reshape(n, C, S)
        K = _bconv(inp, wk, bk).reshape(n, C, S)
        V = _bconv(inp, wv, bv).reshape(n, C, S)
        return Q, K, V

    RQ, RK, RV = qkv(x_re, wq1, bq1, wk1, bk1, wv1, bv1)
    IQ, IK, IV = qkv(y_re, wq2, bq2, wk2, bk2, wv2, bv2)
    DV = (RV.astype(jnp.float32) + IV.astype(jnp.float32)).astype(_BF)

    r1 = _attention(RQ, RK, DV, x, g1, md_up)
    r2 = _attention(IQ, IK, DV, y, g2, md_up)
    r3 = _attention(RQ, IK, RV, y, g3, md_up)
    r4 = _attention(IQ, RK, IV, x, g4, md_up)

    glob = _bconv(jnp.concatenate([r1, r2, r3, r4], axis=1), w_reduce, b_reduce)
    sa_rgb = _spatial_attention(x, w_sa_rgb)
    sa_inf = _spatial_attention(y, w_sa_inf)
    out = _bconv(jnp.concatenate([glob, sa_inf, sa_rgb], axis=1), w_sec, b_sec)
    return out


_compiled = None


def _get_compiled():
    global _compiled
    if _compiled is not None:
        return _compiled
    devs = jax.devices()[:NDEV]
    mesh = Mesh(np.array(devs), ("b",))
    xspec = P("b")
    wspec = P()
    in_specs = (xspec, xspec) + (wspec,) * 22
    fn = jax.jit(
        shard_map(_forward, mesh=mesh, in_specs=in_specs, out_specs=xspec,
                  check_rep=False)
    )
    _compiled = (fn, mesh)
    return _compiled


def kernel(**inputs) -> np.ndarray:
    import ml_dtypes

    fn, mesh = _get_compiled()
    bf = ml_dtypes.bfloat16

    def cast(v):
        return np.asarray(v).astype(bf)

    a = {k: cast(v) for k, v in inputs.items()}
    args = (
        a["x"], a["y"],
        a["w_rgb_q"], a["b_rgb_q"], a["w_rgb_k"], a["b_rgb_k"],
        a["w_rgb_v"], a["b_rgb_v"],
        a["w_inf_q"], a["b_inf_q"], a["w_inf_k"], a["b_inf_k"],
        a["w_inf_v"], a["b_inf_v"],
        a["w_reduce"], a["b_reduce"], a["w_sec"], a["b_sec"],
        a["w_sa_rgb"], a["w_sa_inf"],
        a["gamma1"], a["gamma2"], a["gamma3"], a["gamma4"],
    )
    out = fn(*args)
    return np.asarray(out).astype(np.float32)
